# revision 11
# baseline (speedup 1.0000x reference)
"""Trainium2 Bass kernel for a 2-layer LSTM encoder + autoregressive decoder.

Problem: batch 8192, 48 encode steps, 12 decode steps with BG-channel
feedback, hidden 128, input dim 8, fc head to 1 output.

Strategy: pure data parallelism over 8 NeuronCores (1024 batch rows each).
Per core, the recurrence runs sequentially; within a step everything is
batched with hidden units on SBUF partitions and batch on the free dim:

  gates[512, B] = W_ih^T.T @ x[9, B] + W_hh^T.T @ h[128, B]   (PSUM accum)

so each gate (i, f, g, o) is exactly one 128-partition PSUM tile. Layer 0's
bias is folded into a 9th constant-one input channel; layer 1's bias uses
the activation instruction's per-partition bias operand. The decode BG
feedback is a K=1 accumulating matmul from a [1, B] SBUF tile that is
overwritten with the fc output each step.

Matmuls run in bf16 (fp32 PSUM accumulation); cell state c stays fp32.
"""

import sys

sys.path.insert(0, "/opt/trn_rl_repo")

import numpy as np
import ml_dtypes

import concourse.bacc as bacc
import concourse.tile as tile
from concourse import mybir
from concourse import bass_utils
from concourse.bass import ts

BF16 = ml_dtypes.bfloat16

B_TOTAL = 8192
T = 60
T_ENC = 48
T_DEC = 12
DIN = 8
H = 128
NG = 4 * H  # 512 gate pre-activations per layer
N_CORES = 8
BSH = B_TOTAL // N_CORES  # 1024 batch rows per core
CH = 512  # batch chunk (one PSUM bank wide)
NCH = BSH // CH
XT_STEPS = 8  # timesteps per streamed x tile

# PSUM gate-tile slot -> (pytorch gate index, W column range); slots are
# ordered [i, f, o, g] so the three sigmoid gates are contiguous.
SLOT_GATE = [0, 1, 3, 2]  # slot s holds gate SLOT_GATE[s]; gate g is slot 3

_CACHE: dict = {}


def _build(bfc: float):
    f32 = mybir.dt.float32
    bf16 = mybir.dt.bfloat16

    nc = bacc.Bacc("TRN2", debug=False, num_devices=N_CORES)

    x_d = nc.dram_tensor("x", [DIN + 1, T, BSH], bf16, kind="ExternalInput")
    w9t0_d = nc.dram_tensor("w9t0", [DIN + 1, NG], bf16, kind="ExternalInput")
    whht0_d = nc.dram_tensor("whht0", [H, NG], bf16, kind="ExternalInput")
    wiht1_d = nc.dram_tensor("wiht1", [H, NG], bf16, kind="ExternalInput")
    whht1_d = nc.dram_tensor("whht1", [H, NG], bf16, kind="ExternalInput")
    wbg0t_d = nc.dram_tensor("wbg0t", [1, NG], bf16, kind="ExternalInput")
    wfct_d = nc.dram_tensor("wfct", [H, 1], bf16, kind="ExternalInput")
    b1_d = nc.dram_tensor("b1", [H, 4], f32, kind="ExternalInput")
    bg0_d = nc.dram_tensor("bg0", [1, BSH], bf16, kind="ExternalInput")
    out_d = nc.dram_tensor("out", [T_DEC, BSH], f32, kind="ExternalOutput")

    SIG = mybir.ActivationFunctionType.Sigmoid
    TANH = mybir.ActivationFunctionType.Tanh

    with tile.TileContext(nc) as tc:
        with (
            tc.tile_pool(name="wpool", bufs=1) as wpool,
            tc.tile_pool(name="xpool", bufs=3) as xpool,
            tc.tile_pool(name="state", bufs=1) as state,
            tc.tile_pool(name="gates", bufs=2) as gates,
            tc.tile_pool(name="psum", bufs=2, space="PSUM") as psum,
        ):
            w9t0 = wpool.tile([DIN + 1, NG], bf16)
            whht0 = wpool.tile([H, NG], bf16)
            wiht1 = wpool.tile([H, NG], bf16)
            whht1 = wpool.tile([H, NG], bf16)
            wbg0t = wpool.tile([1, NG], bf16)
            wfct = wpool.tile([H, 1], bf16)
            b1 = wpool.tile([H, 4], f32)
            nc.sync.dma_start(w9t0[:], w9t0_d.ap())
            nc.sync.dma_start(whht0[:], whht0_d.ap())
            nc.sync.dma_start(wiht1[:], wiht1_d.ap())
            nc.sync.dma_start(whht1[:], whht1_d.ap())
            nc.sync.dma_start(wbg0t[:], wbg0t_d.ap())
            nc.sync.dma_start(wfct[:], wfct_d.ap())
            nc.sync.dma_start(b1[:], b1_d.ap())

            h0 = state.tile([H, BSH], bf16)
            h1 = state.tile([H, BSH], bf16)
            c0 = state.tile([H, BSH], f32)
            c1 = state.tile([H, BSH], f32)
            bg = state.tile([1, NCH, CH], bf16)
            nc.vector.memset(h0[:], 0.0)
            nc.vector.memset(h1[:], 0.0)
            nc.vector.memset(c0[:], 0.0)
            nc.vector.memset(c1[:], 0.0)
            nc.sync.dma_start(bg[:], bg0_d.ap())

            xt = None
            for t in range(T):
                if t % XT_STEPS == 0:
                    nt = min(XT_STEPS, T - t)
                    xt = xpool.tile([DIN + 1, XT_STEPS, BSH], bf16)
                    nc.sync.dma_start(
                        xt[:, :nt, :], x_d.ap()[:, t : t + nt, :]
                    )
                tr = t % XT_STEPS
                dec = t >= T_ENC

                for layer in range(2):
                    h_own = h0 if layer == 0 else h1
                    c_own = c0 if layer == 0 else c1
                    w_x = w9t0 if layer == 0 else wiht1
                    w_h = whht0 if layer == 0 else whht1

                    gp = [None] * NCH
                    for c in range(NCH):
                        gps = psum.tile([H, 4, CH], f32, tag="gates")
                        gp[c] = gps
                        if layer == 0:
                            x_sl = xt[:, tr, ts(c, CH)]
                        else:
                            x_sl = h0[:, ts(c, CH)]
                        h_sl = h_own[:, ts(c, CH)]
                        for s in range(4):
                            g = SLOT_GATE[s]
                            cols = ts(g, H)
                            nc.tensor.matmul(
                                gps[:, s, :], w_x[:, cols], x_sl,
                                start=True, stop=False,
                            )
                            if dec and layer == 0:
                                nc.tensor.matmul(
                                    gps[:, s, :], wbg0t[:, cols], bg[:, c, :],
                                    start=False, stop=False,
                                )
                            nc.tensor.matmul(
                                gps[:, s, :], w_h[:, cols], h_sl,
                                start=False, stop=True,
                            )

                    ifo_sb = gates.tile([H, 3, BSH], bf16, tag="ifo")
                    g_sb = gates.tile([H, BSH], bf16, tag="g")
                    for c in range(NCH):
                        gps = gp[c]
                        if layer == 0:
                            nc.scalar.activation(
                                ifo_sb[:, :, ts(c, CH)], gps[:, 0:3, :], SIG
                            )
                            nc.scalar.activation(
                                g_sb[:, ts(c, CH)], gps[:, 3, :], TANH
                            )
                        else:
                            for s in range(3):
                                gi = SLOT_GATE[s]
                                nc.scalar.activation(
                                    ifo_sb[:, s, ts(c, CH)], gps[:, s, :], SIG,
                                    bias=b1[:, gi : gi + 1],
                                )
                            nc.scalar.activation(
                                g_sb[:, ts(c, CH)], gps[:, 3, :], TANH,
                                bias=b1[:, 2:3],
                            )

                    t1 = gates.tile([H, BSH], bf16, tag="t1")
                    u = gates.tile([H, BSH], f32, tag="u")
                    th = gates.tile([H, BSH], bf16, tag="th")
                    nc.vector.tensor_mul(t1[:], ifo_sb[:, 0, :], g_sb[:])
                    nc.vector.tensor_mul(u[:], ifo_sb[:, 1, :], c_own[:])
                    nc.vector.tensor_add(c_own[:], u[:], t1[:])
                    nc.scalar.activation(th[:], c_own[:], TANH)
                    nc.vector.tensor_mul(h_own[:], ifo_sb[:, 2, :], th[:])

                if dec:
                    td = t - T_ENC
                    fc = psum.tile([1, NCH, CH], f32, tag="gates")
                    for c in range(NCH):
                        nc.tensor.matmul(
                            fc[:, c, :], wfct[:], h1[:, ts(c, CH)],
                            start=True, stop=True,
                        )
                    # stage through SBUF (DMA cannot read PSUM), adding b_fc
                    pred = gates.tile([1, NCH, CH], f32, tag="pred")
                    nc.vector.tensor_scalar_add(pred[:], fc[:], bfc)
                    nc.sync.dma_start(out_d.ap()[td : td + 1, :], pred[:])
                    if td + 1 < T_DEC:
                        nc.vector.tensor_copy(bg[:], pred[:])

    nc.compile()
    return nc


def _get_nc(bfc: float):
    if _CACHE.get("bfc") != bfc:
        _CACHE["nc"] = _build(bfc)
        _CACHE["bfc"] = bfc
    return _CACHE["nc"]


def kernel(
    inputs,
    W_ih_0, W_hh_0, b_ih_0, b_hh_0,
    W_ih_1, W_hh_1, b_ih_1, b_hh_1,
    W_fc, b_fc,
):
    inputs = np.asarray(inputs, np.float32)
    bfc = float(np.asarray(b_fc).reshape(-1)[0])
    nc = _get_nc(bfc)

    w9t0 = np.concatenate(
        [W_ih_0.T.astype(np.float32), (b_ih_0 + b_hh_0)[None, :]], axis=0
    ).astype(BF16)  # [9, 512]; row 8 is the bias
    whht0 = W_hh_0.T.astype(BF16)
    wiht1 = W_ih_1.T.astype(BF16)
    whht1 = W_hh_1.T.astype(BF16)
    wbg0t = W_ih_0.T[0:1, :].astype(BF16)  # BG column of W_ih_0
    wfct = W_fc.T.astype(BF16)  # [128, 1]
    b1 = (b_ih_1 + b_hh_1).reshape(4, H).T.astype(np.float32)  # [128, 4]

    in_maps = []
    for i in range(N_CORES):
        sh = inputs[i * BSH : (i + 1) * BSH]  # [1024, 60, 8]
        x = np.ascontiguousarray(sh.transpose(2, 1, 0))  # [8, 60, 1024]
        x9 = np.concatenate(
            [x, np.ones((1, T, BSH), np.float32)], axis=0
        )  # [9, 60, 1024]
        x9[0, T_ENC:, :] = 0.0  # BG channel rides the feedback matmul in decode
        bg0 = sh[:, T_ENC, 0].reshape(1, BSH)
        in_maps.append(
            {
                "x": x9.astype(BF16),
                "w9t0": w9t0,
                "whht0": whht0,
                "wiht1": wiht1,
                "whht1": whht1,
                "wbg0t": wbg0t,
                "wfct": wfct,
                "b1": b1,
                "bg0": bg0.astype(BF16),
            }
        )

    res = bass_utils.run_bass_kernel_spmd(
        nc, in_maps, core_ids=list(range(N_CORES))
    )
    outs = []
    for i in range(N_CORES):
        o = res.results[i]["out"]  # [12, 1024] fp32, b_fc already added
        outs.append(o.T[:, :, None])  # [1024, 12, 1]
    return np.concatenate(outs, axis=0).astype(np.float32)


if __name__ == "__main__":
    _get_nc(0.0)
    print("build + compile OK")


# revision 13
# speedup vs baseline: 1.1538x; 1.1538x over previous
"""Trainium2 Bass kernel for a 2-layer LSTM encoder + autoregressive decoder.

Problem: batch 8192, 48 encode steps, 12 decode steps with BG-channel
feedback, hidden 128, input dim 8, fc head to 1 output.

Strategy: pure data parallelism over 8 NeuronCores (1024 batch rows each).
Per core, the 1024 rows split into two independent 512-row streams whose
recurrences interleave: while one stream runs its activations/elementwise
phase, the other stream's matmuls keep the tensor engine busy (and warm).

Within a step, hidden units sit on SBUF partitions and batch on the free
dim:

  gates[512, B] = W_ih^T.T @ x[9, B] + W_hh^T.T @ h[128, B]   (PSUM accum)

so each gate (i, f, g, o) is exactly one 128-partition PSUM tile. Layer 0's
bias is folded into a 9th constant-one input channel; layer 1's bias uses
the activation instruction's per-partition bias operand. The decode BG
feedback is a K=1 accumulating matmul from a [1, B] SBUF tile that is
overwritten with the fc output each step.

Matmuls run in bf16 (fp32 PSUM accumulation); cell state c stays fp32.
"""

import sys

sys.path.insert(0, "/opt/trn_rl_repo")

import numpy as np
import ml_dtypes

import concourse.bacc as bacc
import concourse.tile as tile
from concourse import mybir
from concourse import bass_utils
from concourse.bass import ts

BF16 = ml_dtypes.bfloat16

B_TOTAL = 8192
T = 60
T_ENC = 48
T_DEC = 12
DIN = 8
H = 128
NG = 4 * H  # 512 gate pre-activations per layer
N_CORES = 8
BSH = B_TOTAL // N_CORES  # 1024 batch rows per core
NS = 2  # independent batch streams per core
SB = BSH // NS  # 512 batch rows per stream (= one PSUM bank)
XT_STEPS = 8  # timesteps per streamed x tile

# PSUM gate-tile slot -> pytorch gate index; slots are ordered [i, f, o, g]
# so the three sigmoid gates are contiguous.
SLOT_GATE = [0, 1, 3, 2]

_CACHE: dict = {}


def _build(bfc: float):
    f32 = mybir.dt.float32
    bf16 = mybir.dt.bfloat16

    nc = bacc.Bacc("TRN2", debug=False, num_devices=N_CORES)

    x_d = nc.dram_tensor("x", [DIN + 1, T, BSH], bf16, kind="ExternalInput")
    w9t0_d = nc.dram_tensor("w9t0", [DIN + 1, NG], bf16, kind="ExternalInput")
    whht0_d = nc.dram_tensor("whht0", [H, NG], bf16, kind="ExternalInput")
    wiht1_d = nc.dram_tensor("wiht1", [H, NG], bf16, kind="ExternalInput")
    whht1_d = nc.dram_tensor("whht1", [H, NG], bf16, kind="ExternalInput")
    wbg0t_d = nc.dram_tensor("wbg0t", [1, NG], bf16, kind="ExternalInput")
    wfct_d = nc.dram_tensor("wfct", [H, 1], bf16, kind="ExternalInput")
    b1_d = nc.dram_tensor("b1", [H, 4], f32, kind="ExternalInput")
    bg0_d = nc.dram_tensor("bg0", [1, BSH], bf16, kind="ExternalInput")
    out_d = nc.dram_tensor("out", [T_DEC, BSH], f32, kind="ExternalOutput")

    SIG = mybir.ActivationFunctionType.Sigmoid
    TANH = mybir.ActivationFunctionType.Tanh

    with tile.TileContext(nc) as tc:
        with (
            tc.tile_pool(name="wpool", bufs=1) as wpool,
            tc.tile_pool(name="xpool", bufs=3) as xpool,
            tc.tile_pool(name="state", bufs=1) as state,
            tc.tile_pool(name="gates", bufs=3) as gates,
            tc.tile_pool(name="psum", bufs=2, space="PSUM") as psum,
        ):
            w9t0 = wpool.tile([DIN + 1, NG], bf16)
            whht0 = wpool.tile([H, NG], bf16)
            wiht1 = wpool.tile([H, NG], bf16)
            whht1 = wpool.tile([H, NG], bf16)
            wbg0t = wpool.tile([1, NG], bf16)
            wfct = wpool.tile([H, 1], bf16)
            b1 = wpool.tile([H, 4], f32)
            nc.sync.dma_start(w9t0[:], w9t0_d.ap())
            nc.sync.dma_start(whht0[:], whht0_d.ap())
            nc.sync.dma_start(wiht1[:], wiht1_d.ap())
            nc.sync.dma_start(whht1[:], whht1_d.ap())
            nc.sync.dma_start(wbg0t[:], wbg0t_d.ap())
            nc.sync.dma_start(wfct[:], wfct_d.ap())
            nc.sync.dma_start(b1[:], b1_d.ap())

            # per-stream recurrent state
            h = [[None] * 2 for _ in range(NS)]  # h[stream][layer]
            c = [[None] * 2 for _ in range(NS)]
            bg = [None] * NS
            for st in range(NS):
                for l in range(2):
                    h[st][l] = state.tile([H, SB], bf16, name=f"h_{st}_{l}")
                    c[st][l] = state.tile([H, SB], f32, name=f"c_{st}_{l}")
                    nc.vector.memset(h[st][l][:], 0.0)
                    nc.vector.memset(c[st][l][:], 0.0)
                bg[st] = state.tile([1, SB], bf16, name=f"bg_{st}")
                nc.sync.dma_start(bg[st][:], bg0_d.ap()[:, ts(st, SB)])

            def layer_block(t, st, layer, xt, tr):
                dec = t >= T_ENC
                h_own = h[st][layer]
                c_own = c[st][layer]
                w_x = w9t0 if layer == 0 else wiht1
                w_h = whht0 if layer == 0 else whht1

                gps = psum.tile([H, 4, SB], f32, tag="gates")
                if layer == 0:
                    x_sl = xt[:, tr, ts(st, SB)]
                else:
                    x_sl = h[st][0][:]
                h_sl = h_own[:]
                for s in range(4):
                    cols = ts(SLOT_GATE[s], H)
                    nc.tensor.matmul(
                        gps[:, s, :], w_x[:, cols], x_sl,
                        start=True, stop=False,
                    )
                    if dec and layer == 0:
                        nc.tensor.matmul(
                            gps[:, s, :], wbg0t[:, cols], bg[st][:],
                            start=False, stop=False,
                        )
                    nc.tensor.matmul(
                        gps[:, s, :], w_h[:, cols], h_sl,
                        start=False, stop=True,
                    )

                ifo_sb = gates.tile([H, 3, SB], bf16, tag="ifo")
                g_sb = gates.tile([H, SB], bf16, tag="g")
                if layer == 0:
                    # bias pre-added via the constant-one input channel
                    nc.scalar.activation(ifo_sb[:], gps[:, 0:3, :], SIG)
                    nc.scalar.activation(g_sb[:], gps[:, 3, :], TANH)
                else:
                    for s in range(3):
                        gi = SLOT_GATE[s]
                        nc.scalar.activation(
                            ifo_sb[:, s, :], gps[:, s, :], SIG,
                            bias=b1[:, gi : gi + 1],
                        )
                    nc.scalar.activation(
                        g_sb[:], gps[:, 3, :], TANH, bias=b1[:, 2:3]
                    )

                t1 = gates.tile([H, SB], bf16, tag="t1")
                u = gates.tile([H, SB], f32, tag="u")
                th = gates.tile([H, SB], bf16, tag="th")
                nc.vector.tensor_mul(t1[:], ifo_sb[:, 0, :], g_sb[:])
                nc.vector.tensor_mul(u[:], ifo_sb[:, 1, :], c_own[:])
                nc.vector.tensor_add(c_own[:], u[:], t1[:])
                nc.scalar.activation(th[:], c_own[:], TANH)
                nc.vector.tensor_mul(h_own[:], ifo_sb[:, 2, :], th[:])

            def fc_block(t, st):
                td = t - T_ENC
                fc = psum.tile([1, SB], f32, tag="gates")
                nc.tensor.matmul(
                    fc[:], wfct[:], h[st][1][:], start=True, stop=True
                )
                # stage through SBUF (DMA cannot read PSUM), adding b_fc
                pred = gates.tile([1, SB], f32, tag="pred")
                nc.vector.tensor_scalar_add(pred[:], fc[:], bfc)
                nc.sync.dma_start(
                    out_d.ap()[td : td + 1, ts(st, SB)], pred[:]
                )
                if td + 1 < T_DEC:
                    nc.vector.tensor_copy(bg[st][:], pred[:])

            xt = None
            for t in range(T):
                if t % XT_STEPS == 0:
                    nt = min(XT_STEPS, T - t)
                    xt = xpool.tile([DIN + 1, XT_STEPS, BSH], bf16)
                    nc.sync.dma_start(
                        xt[:, :nt, :], x_d.ap()[:, t : t + nt, :]
                    )
                tr = t % XT_STEPS
                for st in range(NS):
                    layer_block(t, st, 0, xt, tr)
                for st in range(NS):
                    layer_block(t, st, 1, xt, tr)
                if t >= T_ENC:
                    for st in range(NS):
                        fc_block(t, st)

    nc.compile()
    return nc


def _get_nc(bfc: float):
    if _CACHE.get("bfc") != bfc:
        _CACHE["nc"] = _build(bfc)
        _CACHE["bfc"] = bfc
    return _CACHE["nc"]


def kernel(
    inputs,
    W_ih_0, W_hh_0, b_ih_0, b_hh_0,
    W_ih_1, W_hh_1, b_ih_1, b_hh_1,
    W_fc, b_fc,
):
    inputs = np.asarray(inputs, np.float32)
    bfc = float(np.asarray(b_fc).reshape(-1)[0])
    nc = _get_nc(bfc)

    w9t0 = np.concatenate(
        [W_ih_0.T.astype(np.float32), (b_ih_0 + b_hh_0)[None, :]], axis=0
    ).astype(BF16)  # [9, 512]; row 8 is the bias
    whht0 = W_hh_0.T.astype(BF16)
    wiht1 = W_ih_1.T.astype(BF16)
    whht1 = W_hh_1.T.astype(BF16)
    wbg0t = W_ih_0.T[0:1, :].astype(BF16)  # BG column of W_ih_0
    wfct = W_fc.T.astype(BF16)  # [128, 1]
    b1 = (b_ih_1 + b_hh_1).reshape(4, H).T.astype(np.float32)  # [128, 4]

    in_maps = []
    for i in range(N_CORES):
        sh = inputs[i * BSH : (i + 1) * BSH]  # [1024, 60, 8]
        x = np.ascontiguousarray(sh.transpose(2, 1, 0))  # [8, 60, 1024]
        x9 = np.concatenate(
            [x, np.ones((1, T, BSH), np.float32)], axis=0
        )  # [9, 60, 1024]
        x9[0, T_ENC:, :] = 0.0  # BG channel rides the feedback matmul in decode
        bg0 = sh[:, T_ENC, 0].reshape(1, BSH)
        in_maps.append(
            {
                "x": x9.astype(BF16),
                "w9t0": w9t0,
                "whht0": whht0,
                "wiht1": wiht1,
                "whht1": whht1,
                "wbg0t": wbg0t,
                "wfct": wfct,
                "b1": b1,
                "bg0": bg0.astype(BF16),
            }
        )

    res = bass_utils.run_bass_kernel_spmd(
        nc, in_maps, core_ids=list(range(N_CORES))
    )
    outs = []
    for i in range(N_CORES):
        o = res.results[i]["out"]  # [12, 1024] fp32, b_fc already added
        outs.append(o.T[:, :, None])  # [1024, 12, 1]
    return np.concatenate(outs, axis=0).astype(np.float32)


if __name__ == "__main__":
    _get_nc(0.0)
    print("build + compile OK")
